# revision 14
# baseline (speedup 1.0000x reference)
"""Trainium2 Bass kernel for CustomAttention (qkv -> per-head LN on q,k -> SDPA -> proj).

Sharding: 8 cores = 2 batches x 4 head-groups (3 heads each).
Per core: qkv projection for its heads from x[b], full attention per head,
then a partial output projection over its 192 channels. Host sums the 4
partials per batch and adds proj_b.

Key numerics/scheduling (vs the bf16 baseline):
 - PV matmuls run in fp8-e4m3 DoubleRow mode: V (+ ones column for the
   softmax denominator) is quantized to e4m3 and stored with two adjacent
   128-j chunks in the DoubleRow slot dim, so each PV matmul contracts
   256 j at 0.5 cycles/row -- ~4x fewer PE cycles than the bf16 PV.
 - A v-quantization mean-compensation term (colsum(v - e4m3(v))/N, added
   per-channel after normalize) claws back accuracy.
 - Scores stay bf16, but q is pre-scaled by 8/ln2 so the probability
   tiles can be produced two ways at identical scale: scalar engine
   activation Exp (scale=ln2/8, bias=-2.75, e4m3 out) for 11/16 pairs,
   and a one-op Schraudolph exp on the vector engine (x + magic, max 0,
   uint8 out bitcast as e4m3) for 5/16 pairs -- splitting the 50M-element
   exp stream across two engines. |logit| <= |q||k| = 8 (LN guarantees
   the norms) bounds the uint8 codes to [0, 126], so no saturation edge
   cases.
 - Phase B transposes are packed two heads per [128,128] transpose and
   the old quadrant-pair q duplication is dropped (trace showed quadrant
   matmuls serialize anyway).
"""

import os
import sys
from functools import lru_cache

import numpy as np

for _p in ("/opt/trn_rl_repo", os.path.expanduser("~/.axon_site/_ro/trn_rl_repo")):
    if os.path.isdir(_p) and _p not in sys.path:
        sys.path.insert(0, _p)

import concourse.bass as bass
import concourse.mybir as mybir
from concourse import bacc
import concourse.tile as tile
from concourse.masks import make_identity

F32 = mybir.dt.float32
F32R = mybir.dt.float32r
BF16 = mybir.dt.bfloat16
E4 = mybir.dt.float8e4
U8 = mybir.dt.uint8
ALU = mybir.AluOpType
ACTF = mybir.ActivationFunctionType
AXL = mybir.AxisListType
DR = mybir.MatmulPerfMode.DoubleRow

H = 3          # heads per core
D = 64         # head dim
C = 768        # model dim
J = 3 * H * D  # qkv rows per core = 576
G = 2 * H      # merged LN virtual heads (q0..2, k0..2)
EPS = 1e-5
SCALE = D ** -0.5
SCL = 8.0 / float(np.log(2.0))      # folded into q; scores come out as l*SCL
C_BIAS = 2.75                        # softmax bias: p = exp(l - C_BIAS)
M_SHIFT = 0.4639                     # schraudolph mantissa-correction shift
# f32->uint8 conversion on the DVE rounds to nearest, so no +0.5
B_MAGIC = 56.0 - SCL * C_BIAS - M_SHIFT
SCHR_CPS = frozenset((1, 4, 6, 9, 11, 14))  # pairs routed to the DVE schr-exp
SKEW_PAIRS = 7


def r32(ap):
    return ap.bitcast(F32R)


def build_nc(N=4096):
    """One-core program; all 8 cores run it SPMD with different input data."""
    NB = N // 128          # j-chunks
    CP = NB // 2           # DoubleRow chunk pairs
    IB = N // 512          # i-blocks

    nc = bacc.Bacc("TRN2", target_bir_lowering=False, debug=False)
    x_t = nc.declare_dram_parameter("x_t", [C, N], BF16, isOutput=False)
    wqkv_t = nc.declare_dram_parameter("wqkv_t", [C, J], BF16, isOutput=False)
    projw_t = nc.declare_dram_parameter("projw_t", [H * D, C], F32, isOutput=False)
    # per-partition LN affine columns, rows = d duplicated over both halves:
    # col 0 = gamma_q*scale*SCL, 1 = beta_q*scale*SCL, 2 = gamma_k, 3 = beta_k
    gbc = nc.declare_dram_parameter("gbc", [128, 4], F32, isOutput=False)
    out_p = nc.declare_dram_parameter("out_p", [N, C], F32, isOutput=True)

    with tile.TileContext(nc) as tc:
        with (
            tc.tile_pool(name="persist", bufs=1) as persist,
            tc.tile_pool(name="weights", bufs=1) as weights,
        ):
            # ---- persistent SBUF tensors ----
            # per-head transposed q (duplicated over both partition halves so
            # score matmuls can alternate PE quadrants) and k (j-halves
            # stacked: rows 0:64 = j in [0,N/2), rows 64:128 = j in [N/2,N))
            TQs = [
                persist.tile([128, N], BF16, tag=f"TQ{h}", name=f"TQ{h}")
                for h in range(H)
            ]
            TKs = [
                persist.tile([128, N // 2], BF16, tag=f"TK{h}", name=f"TK{h}")
                for h in range(H)
            ]
            # e4m3 V for DoubleRow PV: [j, head, pair, slot, 80] where cols
            # 0:64 = v, 64 = ones (denominator), 65:80 = zero pad so the
            # slot stride is 16B-aligned. Pair cp covers chunks (cp, cp+16)
            # -- one from each j-half, matching the score quadrant alternation
            vA8 = persist.tile([128, H, CP, 2, 80], E4, tag="vA8")
            # attention output, channel-major: ao1 rows = h0,h1; ao2 rows = h2
            ao1 = persist.tile([128, N], F32R, tag="ao1")
            ao2 = persist.tile([64, N], F32R, tag="ao2")
            # per-channel v-quantization compensation, col h = corr_d/N
            corrT = persist.tile([64, H], F32, tag="corrT")
            onesb = persist.tile([128, 1], BF16, tag="onesb")

            ident = persist.tile([128, 128], F32, tag="ident")
            make_identity(nc, ident[:])
            identb = persist.tile([128, 128], BF16, tag="identb")
            nc.vector.tensor_copy(identb[:], ident[:])
            nc.vector.memset(vA8[:], 0.0)
            nc.vector.memset(vA8[:, :, :, :, 64:65], 1.0)
            nc.vector.memset(onesb[:], 1.0)

            wq = weights.tile([128, 6, J], BF16, tag="wqkv")
            wq_src = wqkv_t.rearrange("(ck p) j -> p ck j", p=128)
            nc.sync.dma_start(wq[:, 0, :], wq_src[:, 0, :])
            gbct = weights.tile([128, 4], F32, tag="gbc")
            epst = weights.tile([128, 1], F32, tag="epst")
            nc.vector.memset(epst[:], EPS)
            cbias = weights.tile([128, 1], F32, tag="cbias")
            nc.vector.memset(cbias[:], -C_BIAS)
            pw128 = weights.tile([128, C], F32R, tag="pw128")
            pw64 = weights.tile([64, C], F32R, tag="pw64")
            corrS = weights.tile([1, H * D], F32, tag="corrS")

            # ================= Phase B: qkv + LN + transpose =================
            # software pipeline, slot s handles: qkv(s), stats(s-1), apply(s-2),
            # transpose+copies(s-3)
            with (
                tc.tile_pool(name="pB", bufs=3) as pB,
                tc.tile_pool(name="pBs", bufs=4) as pBs,
                tc.tile_pool(name="psQ", bufs=2, space="PSUM") as psQ,
                tc.tile_pool(name="psT", bufs=2, space="PSUM") as psT,
                tc.tile_pool(name="psCp", bufs=1, space="PSUM") as psCp,
            ):
                psC = psCp.tile([1, H * D], F32, tag="psC")
                st = {}   # per-nb dict of live tiles

                def emit_qkv(nb):
                    xt = pB.tile([128, 6, 128], BF16, tag="xt")
                    nc.sync.dma_start(
                        xt[:],
                        x_t.rearrange("(ck p) n -> p ck n", p=128)[
                            :, :, nb * 128 : (nb + 1) * 128
                        ],
                    )
                    if nb == 0:
                        # remaining weight slices land while the first x tile
                        # is being consumed
                        for ck in range(1, 6):
                            nc.sync.dma_start(wq[:, ck, :], wq_src[:, ck, :])
                        nc.sync.dma_start(gbct[:], gbc[:, :])
                    # q|k at cols 0:384 (bank 0), v at 512:704 (bank 1)
                    ps = psQ.tile([128, 1024], F32, tag="qkvps")
                    for off, woff, w in ((0, 0, 384), (512, 384, 192)):
                        for ck in range(6):
                            nc.tensor.matmul(
                                ps[:, off : off + w],
                                xt[:, ck, :],
                                wq[:, ck, woff : woff + w],
                                start=(ck == 0),
                                stop=(ck == 5),
                            )
                    st[nb] = {"ps": ps}

                def emit_stats(nb):
                    t = st[nb]
                    ps = t.pop("ps")
                    qkvS = pBs.tile([128, G, D], BF16, tag="qkvS")
                    nc.scalar.copy(
                        qkvS[:], ps[:, 0 : G * D].rearrange("p (g d) -> p g d", d=D)
                    )
                    vsrc = ps[:, 512 : 512 + H * D].rearrange("p (h d) -> p h d", d=D)
                    vdst = vA8[:, :, nb % CP, nb // CP, 0:64]
                    nc.scalar.copy(vdst, vsrc)
                    # v-quantization residual, reduced over j via a ones-matmul
                    rv = pBs.tile([128, H, D], BF16, tag="rv")
                    nc.vector.tensor_sub(rv[:], vsrc, vdst)
                    nc.tensor.matmul(
                        psC[0:1, :], onesb[:, 0:1], rv[:],
                        start=(nb == 0), stop=(nb == NB - 1),
                    )
                    s1 = pBs.tile([128, G], F32, tag="s1")
                    nc.vector.tensor_reduce(s1[:], qkvS[:], AXL.X, ALU.add)
                    mu = pBs.tile([128, G], F32, tag="mu")
                    nc.vector.tensor_scalar_mul(mu[:], s1[:], 1.0 / D)
                    sq = pBs.tile([128, G, D], BF16, tag="sq")
                    nc.vector.tensor_mul(sq[:], qkvS[:], qkvS[:])
                    s2 = pBs.tile([128, G], F32, tag="s2")
                    nc.vector.tensor_reduce(s2[:], sq[:], AXL.X, ALU.add)
                    musq = pBs.tile([128, G], F32, tag="musq")
                    nc.vector.tensor_mul(musq[:], mu[:], mu[:])
                    var = pBs.tile([128, G], F32, tag="var")
                    nc.vector.scalar_tensor_tensor(
                        var[:], s2[:], 1.0 / D, musq[:], ALU.mult, ALU.subtract
                    )
                    std = pBs.tile([128, G], F32, tag="std")
                    nc.scalar.activation(std[:], var[:], ACTF.Sqrt, bias=epst[:])
                    rstd = pBs.tile([128, G], F32, tag="rstd")
                    nc.vector.reciprocal_approx_fast(rstd[:], std[:])
                    t.update(qkvS=qkvS, mu=mu, rstd=rstd)

                def emit_apply(nb):
                    t = st[nb]
                    qkvS, mu, rstd = t.pop("qkvS"), t.pop("mu"), t.pop("rstd")
                    # cs = (x - mu) * rstd on gpsimd; gamma/beta are applied
                    # later, inside the post-transpose per-partition copies
                    cst = pBs.tile([128, G, D], BF16, tag="cst")
                    nc.gpsimd.tensor_sub(
                        cst[:], qkvS[:], mu[:, :, None].broadcast_to([128, G, D])
                    )
                    cs = pBs.tile([128, G, D], BF16, tag="cs")
                    nc.gpsimd.tensor_mul(
                        cs[:], cst[:], rstd[:, :, None].broadcast_to([128, G, D])
                    )
                    t["cs"] = cs

                def emit_transp(nb):
                    t = st[nb]
                    cs = t.pop("cs")
                    jh = nb // CP
                    cb = nb % CP
                    pst = psT.tile([128, 5, 128], BF16, tag="pst")
                    # q0|q1 packed in one transpose; q2 single; k singles land
                    # directly in their j-half rows via tile_position
                    nc.tensor.transpose(pst[:, 0, :], cs[:, 0:2, :], identb[:])
                    nc.tensor.transpose(pst[0:64, 1, :], cs[:, 2, :], identb[:])
                    for hh in range(H):
                        nc.tensor.transpose(
                            pst[64 * jh : 64 * jh + 64, 2 + hh, :],
                            cs[:, 3 + hh, :], identb[:],
                            tile_position=(0, 64 * jh),
                        )
                    blk = slice(nb * 128, (nb + 1) * 128)
                    kblk = slice(cb * 128, (cb + 1) * 128)
                    rows = slice(64 * jh, 64 * jh + 64)
                    # copies apply gamma/beta per partition (= per d after
                    # the transpose): out = in * gamma_col + beta_col
                    nc.scalar.activation(
                        TQs[0][0:64, blk], pst[0:64, 0, :], ACTF.Identity,
                        bias=gbct[0:64, 1:2], scale=gbct[0:64, 0:1],
                    )
                    nc.scalar.activation(
                        TQs[1][64:128, blk], pst[64:128, 0, :], ACTF.Identity,
                        bias=gbct[64:128, 1:2], scale=gbct[64:128, 0:1],
                    )
                    nc.scalar.activation(
                        TQs[2][0:64, blk], pst[0:64, 1, :], ACTF.Identity,
                        bias=gbct[0:64, 1:2], scale=gbct[0:64, 0:1],
                    )
                    for hh in range(H):
                        nc.scalar.activation(
                            TKs[hh][rows, kblk], pst[rows, 2 + hh, :],
                            ACTF.Identity,
                            bias=gbct[rows, 3:4], scale=gbct[rows, 2:3],
                        )
                    # duplicate q into the other partition half
                    nc.sync.dma_start(TQs[0][64:128, blk], TQs[0][0:64, blk])
                    nc.sync.dma_start(TQs[1][0:64, blk], TQs[1][64:128, blk])
                    nc.sync.dma_start(TQs[2][64:128, blk], TQs[2][0:64, blk])
                    del st[nb]

                for s in range(NB + 3):
                    if s == 3:
                        nc.sync.dma_start(pw128[:], projw_t[0:128, :].bitcast(F32R))
                        nc.sync.dma_start(pw64[:], projw_t[128:192, :].bitcast(F32R))
                    if s >= 2 and s - 2 < NB:
                        emit_apply(s - 2)
                    if s >= 3:
                        emit_transp(s - 3)
                    if s < NB:
                        emit_qkv(s)
                    if s >= 1 and s - 1 < NB:
                        emit_stats(s - 1)

                # fold 1/N into the compensation and scatter to per-partition
                # columns (col h = corr for head h's 64 d's)
                nc.vector.tensor_scalar_mul(corrS[:], psC[0:1, :], 1.0 / N)
                for hh in range(H):
                    nc.sync.dma_start(
                        corrT[:, hh : hh + 1], corrS[0:1, hh * D : (hh + 1) * D]
                    )

            # ================= Phase C: attention + proj =================
            with (
                tc.tile_pool(name="pt", bufs=12) as ptp,
                tc.tile_pool(name="pCs", bufs=6) as pCs,
                tc.tile_pool(name="pD", bufs=3) as pD,
                tc.tile_pool(name="psS", bufs=3, space="PSUM") as psS,
                tc.tile_pool(name="psOD", bufs=2, space="PSUM") as psOD,
            ):
                pvq = []      # pending PV closures, global FIFO across heads
                side = []     # pending (ib, proj-unit) from the previous i-block
                ao_done = set()   # i-blocks whose h2 normalize has been emitted

                def pop_pv():
                    pvq.pop(0)()

                def pop_side():
                    if side and side[0][0] in ao_done:
                        side.pop(0)[1]()

                def normalize(ib, h, pso):
                    isl = slice(ib * 512, (ib + 1) * 512)
                    rden_f = pCs.tile([1, 512], F32, tag="rden_f")
                    nc.vector.tensor_copy(rden_f[:], pso[64:65, :])
                    rden = pCs.tile([1, 512], F32, tag="rden")
                    nc.vector.reciprocal_approx_fast(rden[:], rden_f[:])
                    rb = pCs.tile([64, 512], F32, tag="rb")
                    nc.gpsimd.partition_broadcast(rb[:], rden[:])
                    cr = corrT[:, h : h + 1]
                    if h == 0:
                        nc.vector.tensor_mul(ao1[0:64, isl], pso[0:64, :], rb[:])
                        nc.gpsimd.tensor_scalar_add(ao1[0:64, isl], ao1[0:64, isl], cr)
                    elif h == 2:
                        nc.vector.tensor_mul(ao2[0:64, isl], pso[0:64, :], rb[:])
                        nc.gpsimd.tensor_scalar_add(ao2[0:64, isl], ao2[0:64, isl], cr)
                        ao_done.add(ib)
                    else:
                        stg = pCs.tile([64, 512], F32R, tag="stg")
                        nc.vector.tensor_mul(stg[:], pso[0:64, :], rb[:])
                        nc.gpsimd.tensor_scalar_add(stg[:], stg[:], cr)
                        nc.sync.dma_start(ao1[64:128, isl], stg[:])

                def make_pv(pso, h, cp, ib, pt):
                    def run():
                        nc.tensor.matmul(
                            pso,
                            vA8[:, h, cp, :, :],
                            pt[:].rearrange("p (s i) -> p s i", s=2),
                            start=(cp == 0),
                            stop=(cp == CP - 1),
                            perf_mode=DR,
                        )
                        if cp == CP - 1:
                            normalize(ib, h, pso)
                    return run

                def make_proj(ib):
                    units = []
                    for nb in range(ib * 4, ib * 4 + 4):
                        blk = slice(nb * 128, (nb + 1) * 128)
                        stage = [None]

                        def u1(blk=blk, stage=stage):
                            stage[0] = pD.tile([128, C], F32, tag="stage", name="stage")
                            pd_t = psOD.tile([128, 512], F32, tag="psod", name="pd")
                            nc.tensor.matmul(
                                pd_t[:, 0:512], r32(ao1[:, blk]),
                                r32(pw128[:, 0:512]), start=True, stop=False,
                            )
                            nc.tensor.matmul(
                                pd_t[:, 0:512], r32(ao2[0:64, blk]),
                                r32(pw64[0:64, 0:512]), start=False, stop=True,
                            )
                            nc.vector.tensor_copy(stage[0][:, 0:512], pd_t[:, 0:512])

                        def u2(blk=blk, stage=stage):
                            pd_t = psOD.tile([128, 512], F32, tag="psod", name="pd")
                            nc.tensor.matmul(
                                pd_t[:, 0:256], r32(ao1[:, blk]),
                                r32(pw128[:, 512:768]), start=True, stop=False,
                            )
                            nc.tensor.matmul(
                                pd_t[:, 0:256], r32(ao2[0:64, blk]),
                                r32(pw64[0:64, 512:768]), start=False, stop=True,
                            )
                            nc.vector.tensor_copy(stage[0][:, 512:768], pd_t[:, 0:256])
                            nc.sync.dma_start(out_p[blk, :], stage[0][:])

                        units.append((ib, u1))
                        units.append((ib, u2))
                    return units

                for ib in range(IB):
                    isl = slice(ib * 512, (ib + 1) * 512)
                    for h in range(H):
                        TK, TQ = TKs[h], TQs[h]
                        pso_t = psOD.tile([128, 512], F32, tag="psod", name="pso")
                        pso = pso_t[0:80, :]
                        for cp in range(CP):
                            psSp = psS.tile([128, 1024], F32, tag="st")
                            # pair = chunks (cp, cp+16): one from each j-half,
                            # so consecutive score matmuls alternate quadrants
                            # and LDWEIGHTS hides behind the running stream
                            kblk = slice(cp * 128, (cp + 1) * 128)
                            nc.tensor.matmul(
                                psSp[:, 0:512], TK[0:64, kblk], TQ[0:64, isl],
                                start=True, stop=True, tile_position=(0, 0),
                            )
                            if len(pvq) > SKEW_PAIRS:
                                pop_pv()
                            if cp % 2 == 0:
                                pop_side()
                            nc.tensor.matmul(
                                psSp[:, 512:1024], TK[64:128, kblk], TQ[64:128, isl],
                                start=True, stop=True, tile_position=(64, 0),
                            )
                            if len(pvq) > SKEW_PAIRS + 1:
                                pop_pv()
                            pt = ptp.tile([128, 1024], E4, tag="pt")
                            if cp in SCHR_CPS:
                                nc.vector.tensor_scalar(
                                    pt[:].bitcast(U8), psSp[:], B_MAGIC, 0.0,
                                    ALU.add, ALU.max,
                                )
                            else:
                                nc.scalar.activation(
                                    pt[:], psSp[:], ACTF.Exp,
                                    bias=cbias[:], scale=1.0 / SCL,
                                )
                            pvq.append(make_pv(pso, h, cp, ib, pt))
                    # queue this i-block's projection for the next i-block
                    side.extend(make_proj(ib))
                    if ib == IB - 1:
                        while pvq:
                            pop_pv()
                        while side:
                            side.pop(0)[1]()

    nc.compile()
    return nc


@lru_cache(maxsize=2)
def _built(N):
    nc = build_nc(N)
    return nc


def _prep_inputs(x, qkv_w, q_gamma, q_beta, k_gamma, k_beta, proj_w):
    x = np.asarray(x, np.float32)
    qkv_w = np.asarray(qkv_w, np.float32)
    proj_w = np.asarray(proj_w, np.float32)
    B = x.shape[0]
    import ml_dtypes
    xts = [np.ascontiguousarray(x[b].T).astype(ml_dtypes.bfloat16) for b in range(B)]
    qsc = SCALE * SCL
    gb2 = np.stack(
        [
            np.tile(np.asarray(q_gamma, np.float32) * qsc, 2),
            np.tile(np.asarray(q_beta, np.float32) * qsc, 2),
            np.tile(np.asarray(k_gamma, np.float32), 2),
            np.tile(np.asarray(k_beta, np.float32), 2),
        ],
        axis=1,
    )  # [128, 4]
    gbs = []
    wqs = []
    pws = []
    for g in range(4):
        r = slice(192 * g, 192 * (g + 1))
        wq_rows = np.concatenate(
            [qkv_w[r], qkv_w[768:1536][r], qkv_w[1536:2304][r]], axis=0
        )
        wqs.append(np.ascontiguousarray(wq_rows.T).astype(ml_dtypes.bfloat16))
        pws.append(np.ascontiguousarray(proj_w[:, r].T))
        gbs.append(gb2)
    in_maps = []
    for core in range(8):
        b, g = core // 4, core % 4
        in_maps.append(
            {"x_t": xts[b], "wqkv_t": wqs[g], "projw_t": pws[g], "gbc": gbs[g]}
        )
    return in_maps


def run_cores(in_maps, N, trace=False):
    from concourse.bass_utils import run_bass_kernel_spmd

    nc = _built(N)
    res = run_bass_kernel_spmd(nc, in_maps, list(range(8)), trace=trace)
    return res


def kernel(x, qkv_w, q_gamma, q_beta, k_gamma, k_beta, proj_w, proj_b):
    x = np.asarray(x, np.float32)
    N = x.shape[1]
    in_maps = _prep_inputs(x, qkv_w, q_gamma, q_beta, k_gamma, k_beta, proj_w)
    res = run_cores(in_maps, N)
    parts = [np.asarray(r["out_p"], np.float32) for r in res.results]
    out0 = parts[0] + parts[1] + parts[2] + parts[3]
    out1 = parts[4] + parts[5] + parts[6] + parts[7]
    out = np.stack([out0, out1]) + np.asarray(proj_b, np.float32)
    return out.astype(np.float32)


# revision 16
# speedup vs baseline: 1.1865x; 1.1865x over previous
"""Trainium2 Bass kernel for CustomAttention (qkv -> per-head LN on q,k -> SDPA -> proj).

Sharding: 8 cores = 2 batches x 4 head-groups (3 heads each).
Per core: qkv projection for its heads from x[b], full attention per head,
then a partial output projection over its 192 channels. Host sums the 4
partials per batch and adds proj_b.

Key numerics/scheduling (vs the bf16 baseline):
 - PV matmuls run in fp8-e4m3 DoubleRow mode: V (+ ones column for the
   softmax denominator) is quantized to e4m3 and stored with two adjacent
   128-j chunks in the DoubleRow slot dim, so each PV matmul contracts
   256 j at 0.5 cycles/row -- ~4x fewer PE cycles than the bf16 PV.
 - A v-quantization mean-compensation term (colsum(v - e4m3(v))/N, added
   per-channel after normalize) claws back accuracy.
 - Scores stay bf16, but q is pre-scaled by 8/ln2 so the probability
   tiles can be produced two ways at identical scale: scalar engine
   activation Exp (scale=ln2/8, bias=-2.75, e4m3 out) for 11/16 pairs,
   and a one-op Schraudolph exp on the vector engine (x + magic, max 0,
   uint8 out bitcast as e4m3) for 5/16 pairs -- splitting the 50M-element
   exp stream across two engines. |logit| <= |q||k| = 8 (LN guarantees
   the norms) bounds the uint8 codes to [0, 126], so no saturation edge
   cases.
 - Phase B transposes are packed two heads per [128,128] transpose and
   the old quadrant-pair q duplication is dropped (trace showed quadrant
   matmuls serialize anyway).
"""

import os
import sys
from functools import lru_cache

import numpy as np

for _p in ("/opt/trn_rl_repo", os.path.expanduser("~/.axon_site/_ro/trn_rl_repo")):
    if os.path.isdir(_p) and _p not in sys.path:
        sys.path.insert(0, _p)

import concourse.bass as bass
import concourse.mybir as mybir
from concourse import bacc
import concourse.tile as tile
from concourse.masks import make_identity

F32 = mybir.dt.float32
F32R = mybir.dt.float32r
BF16 = mybir.dt.bfloat16
E4 = mybir.dt.float8e4
U8 = mybir.dt.uint8
ALU = mybir.AluOpType
ACTF = mybir.ActivationFunctionType
AXL = mybir.AxisListType
DR = mybir.MatmulPerfMode.DoubleRow

H = 3          # heads per core
D = 64         # head dim
C = 768        # model dim
J = 3 * H * D  # qkv rows per core = 576
G = 2 * H      # merged LN virtual heads (q0..2, k0..2)
EPS = 1e-5
SCALE = D ** -0.5
SCL = 8.0 / float(np.log(2.0))      # folded into q; scores come out as l*SCL
C_BIAS = 2.75                        # softmax bias: p = exp(l - C_BIAS)
M_SHIFT = 0.4639                     # schraudolph mantissa-correction shift
# f32->uint8 conversion on the DVE rounds to nearest, so no +0.5
B_MAGIC = 56.0 - SCL * C_BIAS - M_SHIFT
SCHR_CPS = frozenset((1, 4, 6, 9, 11, 14))  # pairs routed to the DVE schr-exp
SKEW_PAIRS = 7


def r32(ap):
    return ap.bitcast(F32R)


def build_nc(N=4096):
    """One-core program; all 8 cores run it SPMD with different input data."""
    NB = N // 128          # j-chunks
    CP = NB // 2           # DoubleRow chunk pairs
    IB = N // 512          # i-blocks

    nc = bacc.Bacc("TRN2", target_bir_lowering=False, debug=False)
    x_t = nc.declare_dram_parameter("x_t", [C, N], BF16, isOutput=False)
    wqkv_t = nc.declare_dram_parameter("wqkv_t", [C, J], BF16, isOutput=False)
    projw_t = nc.declare_dram_parameter("projw_t", [H * D, C], F32, isOutput=False)
    # per-partition LN affine columns, rows = d duplicated over both halves:
    # col 0 = gamma_q*scale*SCL, 1 = beta_q*scale*SCL, 2 = gamma_k, 3 = beta_k
    gbc = nc.declare_dram_parameter("gbc", [128, 4], F32, isOutput=False)
    out_p = nc.declare_dram_parameter("out_p", [N, C], F32, isOutput=True)

    with tile.TileContext(nc) as tc:
        with (
            tc.tile_pool(name="persist", bufs=1) as persist,
            tc.tile_pool(name="weights", bufs=1) as weights,
        ):
            # ---- persistent SBUF tensors ----
            # per-head transposed q (duplicated over both partition halves so
            # score matmuls can alternate PE quadrants) and k (j-halves
            # stacked: rows 0:64 = j in [0,N/2), rows 64:128 = j in [N/2,N))
            TQs = [
                persist.tile([128, N], BF16, tag=f"TQ{h}", name=f"TQ{h}")
                for h in range(H)
            ]
            TKs = [
                persist.tile([128, N // 2], BF16, tag=f"TK{h}", name=f"TK{h}")
                for h in range(H)
            ]
            # e4m3 V for DoubleRow PV: [j, head, pair, slot, 80] where cols
            # 0:64 = v, 64 = ones (denominator), 65:80 = zero pad so the
            # slot stride is 16B-aligned. Pair cp covers chunks (cp, cp+16)
            # -- one from each j-half, matching the score quadrant alternation
            vA8 = persist.tile([128, H, CP, 2, 80], E4, tag="vA8")
            # attention output, channel-major: ao1 rows = h0,h1; ao2 rows = h2
            ao1 = persist.tile([128, N], F32R, tag="ao1")
            ao2 = persist.tile([64, N], F32R, tag="ao2")
            # per-channel v-quantization compensation, col h = corr_d/N
            corrT = persist.tile([64, H], F32, tag="corrT")
            onesb = persist.tile([128, 1], BF16, tag="onesb")

            ident = persist.tile([128, 128], F32, tag="ident")
            make_identity(nc, ident[:])
            identb = persist.tile([128, 128], BF16, tag="identb")
            nc.vector.tensor_copy(identb[:], ident[:])
            nc.vector.memset(vA8[:], 0.0)
            nc.vector.memset(vA8[:, :, :, :, 64:65], 1.0)
            nc.vector.memset(onesb[:], 1.0)

            wq = weights.tile([128, 6, J], BF16, tag="wqkv")
            wq_src = wqkv_t.rearrange("(ck p) j -> p ck j", p=128)
            nc.sync.dma_start(wq[:, 0, :], wq_src[:, 0, :])
            gbct = weights.tile([128, 4], F32, tag="gbc")
            epst = weights.tile([128, 1], F32, tag="epst")
            nc.vector.memset(epst[:], EPS)
            cbias = weights.tile([128, 1], F32, tag="cbias")
            nc.vector.memset(cbias[:], -C_BIAS)
            pw128 = weights.tile([128, C], F32R, tag="pw128")
            pw64 = weights.tile([64, C], F32R, tag="pw64")
            corrS = weights.tile([1, H * D], F32, tag="corrS")

            # ================= Phase B: qkv + LN + transpose =================
            # software pipeline, slot s handles: qkv(s), stats(s-1), apply(s-2),
            # transpose+copies(s-3)
            with (
                tc.tile_pool(name="pB", bufs=3) as pB,
                tc.tile_pool(name="pBs", bufs=4) as pBs,
                tc.tile_pool(name="psQ", bufs=2, space="PSUM") as psQ,
                tc.tile_pool(name="psT", bufs=2, space="PSUM") as psT,
                tc.tile_pool(name="psCp", bufs=1, space="PSUM") as psCp,
            ):
                psC = psCp.tile([1, H * D], F32, tag="psC")
                st = {}   # per-nb dict of live tiles

                def emit_qkv(nb):
                    xt = pB.tile([128, 6, 128], BF16, tag="xt")
                    nc.sync.dma_start(
                        xt[:],
                        x_t.rearrange("(ck p) n -> p ck n", p=128)[
                            :, :, nb * 128 : (nb + 1) * 128
                        ],
                    )
                    if nb == 0:
                        # remaining weight slices land while the first x tile
                        # is being consumed
                        for ck in range(1, 6):
                            nc.sync.dma_start(wq[:, ck, :], wq_src[:, ck, :])
                        nc.sync.dma_start(gbct[:], gbc[:, :])
                    # q|k at cols 0:384 (bank 0), v at 512:704 (bank 1)
                    ps = psQ.tile([128, 1024], F32, tag="qkvps")
                    for off, woff, w in ((0, 0, 384), (512, 384, 192)):
                        for ck in range(6):
                            nc.tensor.matmul(
                                ps[:, off : off + w],
                                xt[:, ck, :],
                                wq[:, ck, woff : woff + w],
                                start=(ck == 0),
                                stop=(ck == 5),
                            )
                    st[nb] = {"ps": ps}

                def emit_stats(nb):
                    t = st[nb]
                    ps = t.pop("ps")
                    qkvS = pBs.tile([128, G, D], BF16, tag="qkvS")
                    nc.scalar.copy(
                        qkvS[:], ps[:, 0 : G * D].rearrange("p (g d) -> p g d", d=D)
                    )
                    vsrc = ps[:, 512 : 512 + H * D].rearrange("p (h d) -> p h d", d=D)
                    vdst = vA8[:, :, nb % CP, nb // CP, 0:64]
                    nc.scalar.copy(vdst, vsrc)
                    # v-quantization residual, reduced over j via a ones-matmul
                    rv = pBs.tile([128, H, D], BF16, tag="rv")
                    nc.vector.tensor_sub(rv[:], vsrc, vdst)
                    nc.tensor.matmul(
                        psC[0:1, :], onesb[:, 0:1], rv[:],
                        start=(nb == 0), stop=(nb == NB - 1),
                    )
                    s1 = pBs.tile([128, G], F32, tag="s1")
                    nc.vector.tensor_reduce(s1[:], qkvS[:], AXL.X, ALU.add)
                    mu = pBs.tile([128, G], F32, tag="mu")
                    nc.vector.tensor_scalar_mul(mu[:], s1[:], 1.0 / D)
                    sq = pBs.tile([128, G, D], BF16, tag="sq")
                    nc.vector.tensor_mul(sq[:], qkvS[:], qkvS[:])
                    s2 = pBs.tile([128, G], F32, tag="s2")
                    nc.vector.tensor_reduce(s2[:], sq[:], AXL.X, ALU.add)
                    musq = pBs.tile([128, G], F32, tag="musq")
                    nc.vector.tensor_mul(musq[:], mu[:], mu[:])
                    var = pBs.tile([128, G], F32, tag="var")
                    nc.vector.scalar_tensor_tensor(
                        var[:], s2[:], 1.0 / D, musq[:], ALU.mult, ALU.subtract
                    )
                    std = pBs.tile([128, G], F32, tag="std")
                    nc.scalar.activation(std[:], var[:], ACTF.Sqrt, bias=epst[:])
                    rstd = pBs.tile([128, G], F32, tag="rstd")
                    nc.vector.reciprocal_approx_fast(rstd[:], std[:])
                    t.update(qkvS=qkvS, mu=mu, rstd=rstd)

                def emit_apply(nb):
                    t = st[nb]
                    qkvS, mu, rstd = t.pop("qkvS"), t.pop("mu"), t.pop("rstd")
                    # cs = (x - mu) * rstd on gpsimd; gamma/beta are applied
                    # later, inside the post-transpose per-partition copies
                    cst = pBs.tile([128, G, D], BF16, tag="cst")
                    nc.gpsimd.tensor_sub(
                        cst[:], qkvS[:], mu[:, :, None].broadcast_to([128, G, D])
                    )
                    cs = pBs.tile([128, G, D], BF16, tag="cs")
                    nc.gpsimd.tensor_mul(
                        cs[:], cst[:], rstd[:, :, None].broadcast_to([128, G, D])
                    )
                    t["cs"] = cs

                def emit_transp(nb):
                    t = st[nb]
                    cs = t.pop("cs")
                    jh = nb // CP
                    cb = nb % CP
                    pst = psT.tile([128, 5, 128], BF16, tag="pst")
                    # q0|q1 packed in one transpose; q2 single; k singles land
                    # directly in their j-half rows via tile_position
                    nc.tensor.transpose(pst[:, 0, :], cs[:, 0:2, :], identb[:])
                    nc.tensor.transpose(pst[0:64, 1, :], cs[:, 2, :], identb[:])
                    for hh in range(H):
                        nc.tensor.transpose(
                            pst[64 * jh : 64 * jh + 64, 2 + hh, :],
                            cs[:, 3 + hh, :], identb[:],
                            tile_position=(0, 64 * jh),
                        )
                    blk = slice(nb * 128, (nb + 1) * 128)
                    kblk = slice(cb * 128, (cb + 1) * 128)
                    rows = slice(64 * jh, 64 * jh + 64)
                    # copies apply gamma/beta per partition (= per d after
                    # the transpose): out = in * gamma_col + beta_col
                    nc.scalar.activation(
                        TQs[0][0:64, blk], pst[0:64, 0, :], ACTF.Identity,
                        bias=gbct[0:64, 1:2], scale=gbct[0:64, 0:1],
                    )
                    nc.scalar.activation(
                        TQs[1][64:128, blk], pst[64:128, 0, :], ACTF.Identity,
                        bias=gbct[64:128, 1:2], scale=gbct[64:128, 0:1],
                    )
                    nc.scalar.activation(
                        TQs[2][0:64, blk], pst[0:64, 1, :], ACTF.Identity,
                        bias=gbct[0:64, 1:2], scale=gbct[0:64, 0:1],
                    )
                    for hh in range(H):
                        nc.scalar.activation(
                            TKs[hh][rows, kblk], pst[rows, 2 + hh, :],
                            ACTF.Identity,
                            bias=gbct[rows, 3:4], scale=gbct[rows, 2:3],
                        )
                    # duplicate q into the other partition half
                    nc.sync.dma_start(TQs[0][64:128, blk], TQs[0][0:64, blk])
                    nc.sync.dma_start(TQs[1][0:64, blk], TQs[1][64:128, blk])
                    nc.sync.dma_start(TQs[2][64:128, blk], TQs[2][0:64, blk])
                    del st[nb]

                for s in range(NB + 3):
                    if s == 3:
                        nc.sync.dma_start(pw128[:], projw_t[0:128, :].bitcast(F32R))
                        nc.sync.dma_start(pw64[:], projw_t[128:192, :].bitcast(F32R))
                    if s >= 2 and s - 2 < NB:
                        emit_apply(s - 2)
                    if s >= 3:
                        emit_transp(s - 3)
                    if s < NB:
                        emit_qkv(s)
                    if s >= 1 and s - 1 < NB:
                        emit_stats(s - 1)

                # fold 1/N into the compensation and scatter to per-partition
                # columns (col h = corr for head h's 64 d's)
                nc.vector.tensor_scalar_mul(corrS[:], psC[0:1, :], 1.0 / N)
                for hh in range(H):
                    nc.sync.dma_start(
                        corrT[:, hh : hh + 1], corrS[0:1, hh * D : (hh + 1) * D]
                    )

            # ================= Phase C: attention + proj =================
            with (
                tc.tile_pool(name="pt", bufs=12) as ptp,
                tc.tile_pool(name="pCs", bufs=6) as pCs,
                tc.tile_pool(name="pD", bufs=3) as pD,
                tc.tile_pool(name="psS", bufs=3, space="PSUM") as psS,
                tc.tile_pool(name="psOD", bufs=2, space="PSUM") as psOD,
            ):
                pvq = []      # pending PV closures, global FIFO across heads
                side = []     # pending (ib, proj-unit) from the previous i-block
                ao_done = set()   # i-blocks whose h2 normalize has been emitted

                def pop_pv():
                    pvq.pop(0)()

                def pop_side():
                    if side and side[0][0] in ao_done:
                        side.pop(0)[1]()

                def normalize(ib, h, pso):
                    isl = slice(ib * 512, (ib + 1) * 512)
                    rden_f = pCs.tile([1, 512], F32, tag="rden_f")
                    nc.vector.tensor_copy(rden_f[:], pso[64:65, :])
                    rden = pCs.tile([1, 512], F32, tag="rden")
                    nc.vector.reciprocal_approx_fast(rden[:], rden_f[:])
                    rb = pCs.tile([64, 512], F32, tag="rb")
                    nc.gpsimd.partition_broadcast(rb[:], rden[:])
                    cr = corrT[:, h : h + 1]
                    if h == 0:
                        nc.vector.tensor_mul(ao1[0:64, isl], pso[0:64, :], rb[:])
                        nc.vector.tensor_scalar_add(ao1[0:64, isl], ao1[0:64, isl], cr)
                    elif h == 2:
                        nc.vector.tensor_mul(ao2[0:64, isl], pso[0:64, :], rb[:])
                        nc.vector.tensor_scalar_add(ao2[0:64, isl], ao2[0:64, isl], cr)
                        ao_done.add(ib)
                    else:
                        stg = pCs.tile([64, 512], F32R, tag="stg")
                        nc.vector.tensor_mul(stg[:], pso[0:64, :], rb[:])
                        nc.vector.tensor_scalar_add(stg[:], stg[:], cr)
                        nc.sync.dma_start(ao1[64:128, isl], stg[:])

                def make_pv(pso, h, cp, ib, pt):
                    def run():
                        nc.tensor.matmul(
                            pso,
                            vA8[:, h, cp, :, :],
                            pt[:].rearrange("p (s i) -> p s i", s=2),
                            start=(cp == 0),
                            stop=(cp == CP - 1),
                            perf_mode=DR,
                        )
                        if cp == CP - 1:
                            normalize(ib, h, pso)
                    return run

                def make_proj(ib):
                    units = []
                    for nb in range(ib * 4, ib * 4 + 4):
                        blk = slice(nb * 128, (nb + 1) * 128)
                        stage = [None]

                        def u1(blk=blk, stage=stage):
                            stage[0] = pD.tile([128, C], F32, tag="stage", name="stage")
                            pd_t = psOD.tile([128, 512], F32, tag="psod", name="pd")
                            nc.tensor.matmul(
                                pd_t[:, 0:512], r32(ao1[:, blk]),
                                r32(pw128[:, 0:512]), start=True, stop=False,
                            )
                            nc.tensor.matmul(
                                pd_t[:, 0:512], r32(ao2[0:64, blk]),
                                r32(pw64[0:64, 0:512]), start=False, stop=True,
                            )
                            nc.vector.tensor_copy(stage[0][:, 0:512], pd_t[:, 0:512])

                        def u2(blk=blk, stage=stage):
                            pd_t = psOD.tile([128, 512], F32, tag="psod", name="pd")
                            nc.tensor.matmul(
                                pd_t[:, 0:256], r32(ao1[:, blk]),
                                r32(pw128[:, 512:768]), start=True, stop=False,
                            )
                            nc.tensor.matmul(
                                pd_t[:, 0:256], r32(ao2[0:64, blk]),
                                r32(pw64[0:64, 512:768]), start=False, stop=True,
                            )
                            nc.vector.tensor_copy(stage[0][:, 512:768], pd_t[:, 0:256])
                            nc.sync.dma_start(out_p[blk, :], stage[0][:])

                        units.append((ib, u1))
                        units.append((ib, u2))
                    return units

                for ib in range(IB):
                    isl = slice(ib * 512, (ib + 1) * 512)
                    for h in range(H):
                        TK, TQ = TKs[h], TQs[h]
                        pso_t = psOD.tile([128, 512], F32, tag="psod", name="pso")
                        pso = pso_t[0:80, :]
                        for cp in range(CP):
                            psSp = psS.tile([128, 1024], F32, tag="st")
                            # pair = chunks (cp, cp+16): one from each j-half,
                            # so consecutive score matmuls alternate quadrants
                            # and LDWEIGHTS hides behind the running stream
                            kblk = slice(cp * 128, (cp + 1) * 128)
                            nc.tensor.matmul(
                                psSp[:, 0:512], TK[0:64, kblk], TQ[0:64, isl],
                                start=True, stop=True, tile_position=(0, 0),
                            )
                            if len(pvq) > SKEW_PAIRS:
                                pop_pv()
                            if cp % 2 == 0:
                                pop_side()
                            nc.tensor.matmul(
                                psSp[:, 512:1024], TK[64:128, kblk], TQ[64:128, isl],
                                start=True, stop=True, tile_position=(64, 0),
                            )
                            if len(pvq) > SKEW_PAIRS + 1:
                                pop_pv()
                            pt = ptp.tile([128, 1024], E4, tag="pt")
                            if cp in SCHR_CPS:
                                nc.vector.tensor_scalar(
                                    pt[:].bitcast(U8), psSp[:], B_MAGIC, 0.0,
                                    ALU.add, ALU.max,
                                )
                            else:
                                nc.scalar.activation(
                                    pt[:], psSp[:], ACTF.Exp,
                                    bias=cbias[:], scale=1.0 / SCL,
                                )
                            pvq.append(make_pv(pso, h, cp, ib, pt))
                    # queue this i-block's projection for the next i-block
                    side.extend(make_proj(ib))
                    if ib == IB - 1:
                        while pvq:
                            pop_pv()
                        while side:
                            side.pop(0)[1]()

    nc.compile()
    return nc


@lru_cache(maxsize=2)
def _built(N):
    nc = build_nc(N)
    return nc


def _prep_inputs(x, qkv_w, q_gamma, q_beta, k_gamma, k_beta, proj_w):
    x = np.asarray(x, np.float32)
    qkv_w = np.asarray(qkv_w, np.float32)
    proj_w = np.asarray(proj_w, np.float32)
    B = x.shape[0]
    import ml_dtypes
    xts = [np.ascontiguousarray(x[b].T).astype(ml_dtypes.bfloat16) for b in range(B)]
    qsc = SCALE * SCL
    gb2 = np.stack(
        [
            np.tile(np.asarray(q_gamma, np.float32) * qsc, 2),
            np.tile(np.asarray(q_beta, np.float32) * qsc, 2),
            np.tile(np.asarray(k_gamma, np.float32), 2),
            np.tile(np.asarray(k_beta, np.float32), 2),
        ],
        axis=1,
    )  # [128, 4]
    gbs = []
    wqs = []
    pws = []
    for g in range(4):
        r = slice(192 * g, 192 * (g + 1))
        wq_rows = np.concatenate(
            [qkv_w[r], qkv_w[768:1536][r], qkv_w[1536:2304][r]], axis=0
        )
        wqs.append(np.ascontiguousarray(wq_rows.T).astype(ml_dtypes.bfloat16))
        pws.append(np.ascontiguousarray(proj_w[:, r].T))
        gbs.append(gb2)
    in_maps = []
    for core in range(8):
        b, g = core // 4, core % 4
        in_maps.append(
            {"x_t": xts[b], "wqkv_t": wqs[g], "projw_t": pws[g], "gbc": gbs[g]}
        )
    return in_maps


def run_cores(in_maps, N, trace=False):
    from concourse.bass_utils import run_bass_kernel_spmd

    nc = _built(N)
    res = run_bass_kernel_spmd(nc, in_maps, list(range(8)), trace=trace)
    return res


def kernel(x, qkv_w, q_gamma, q_beta, k_gamma, k_beta, proj_w, proj_b):
    x = np.asarray(x, np.float32)
    N = x.shape[1]
    in_maps = _prep_inputs(x, qkv_w, q_gamma, q_beta, k_gamma, k_beta, proj_w)
    res = run_cores(in_maps, N)
    parts = [np.asarray(r["out_p"], np.float32) for r in res.results]
    out0 = parts[0] + parts[1] + parts[2] + parts[3]
    out1 = parts[4] + parts[5] + parts[6] + parts[7]
    out = np.stack([out0, out1]) + np.asarray(proj_b, np.float32)
    return out.astype(np.float32)


# revision 22
# speedup vs baseline: 1.2015x; 1.0126x over previous
"""Trainium2 Bass kernel for CustomAttention (qkv -> per-head LN on q,k -> SDPA -> proj).

Sharding: 8 cores = 2 batches x 4 head-groups (3 heads each).
Per core: qkv projection for its heads from x[b], full attention per head,
then a partial output projection over its 192 channels. Host sums the 4
partials per batch and adds proj_b.

Key numerics/scheduling (vs the bf16 baseline):
 - PV matmuls run in fp8-e4m3 DoubleRow mode: V (+ ones column for the
   softmax denominator) is quantized to e4m3 and stored with two adjacent
   128-j chunks in the DoubleRow slot dim, so each PV matmul contracts
   256 j at 0.5 cycles/row -- ~4x fewer PE cycles than the bf16 PV.
 - A v-quantization mean-compensation term (colsum(v - e4m3(v))/N, added
   per-channel after normalize) claws back accuracy.
 - Scores stay bf16, but q is pre-scaled by 8/ln2 so the probability
   tiles can be produced two ways at identical scale: scalar engine
   activation Exp (scale=ln2/8, bias=-2.75, e4m3 out) for 11/16 pairs,
   and a one-op Schraudolph exp on the vector engine (x + magic, max 0,
   uint8 out bitcast as e4m3) for 5/16 pairs -- splitting the 50M-element
   exp stream across two engines. |logit| <= |q||k| = 8 (LN guarantees
   the norms) bounds the uint8 codes to [0, 126], so no saturation edge
   cases.
 - Phase B transposes are packed two heads per [128,128] transpose and
   the old quadrant-pair q duplication is dropped (trace showed quadrant
   matmuls serialize anyway).
"""

import os
import sys
from functools import lru_cache

import numpy as np

for _p in ("/opt/trn_rl_repo", os.path.expanduser("~/.axon_site/_ro/trn_rl_repo")):
    if os.path.isdir(_p) and _p not in sys.path:
        sys.path.insert(0, _p)

import concourse.bass as bass
import concourse.mybir as mybir
from concourse import bacc
import concourse.tile as tile
from concourse.masks import make_identity

F32 = mybir.dt.float32
F32R = mybir.dt.float32r
BF16 = mybir.dt.bfloat16
E4 = mybir.dt.float8e4
U8 = mybir.dt.uint8
ALU = mybir.AluOpType
ACTF = mybir.ActivationFunctionType
AXL = mybir.AxisListType
DR = mybir.MatmulPerfMode.DoubleRow

H = 3          # heads per core
D = 64         # head dim
C = 768        # model dim
J = 3 * H * D  # qkv rows per core = 576
G = 2 * H      # merged LN virtual heads (q0..2, k0..2)
EPS = 1e-5
SCALE = D ** -0.5
SCL = 8.0 / float(np.log(2.0))      # folded into q; scores come out as l*SCL
C_BIAS = 2.75                        # softmax bias: p = exp(l - C_BIAS)
M_SHIFT = 0.4639                     # schraudolph mantissa-correction shift
# f32->uint8 conversion on the DVE rounds to nearest, so no +0.5
B_MAGIC = 56.0 - SCL * C_BIAS - M_SHIFT
SCHR_CPS = frozenset((1, 4, 7, 10, 13))  # pairs routed to the DVE schr-exp
QI = (0, 2, 1)  # head h lives at TQall[:, QI[h], :] (q2 packed next to q0)
SKEW_PAIRS = 7


def r32(ap):
    return ap.bitcast(F32R)


def build_nc(N=4096):
    """One-core program; all 8 cores run it SPMD with different input data."""
    NB = N // 128          # j-chunks
    CP = NB // 2           # DoubleRow chunk pairs
    IB = N // 512          # i-blocks

    nc = bacc.Bacc("TRN2", target_bir_lowering=False, debug=False)
    x_t = nc.declare_dram_parameter("x_t", [C, N], BF16, isOutput=False)
    wqkv_t = nc.declare_dram_parameter("wqkv_t", [C, J], BF16, isOutput=False)
    projw_t = nc.declare_dram_parameter("projw_t", [H * D, C], F32, isOutput=False)
    # per-partition LN affine columns, rows = d duplicated over both halves:
    # col 0 = gamma_q*scale*SCL, 1 = beta_q*scale*SCL, 2 = gamma_k, 3 = beta_k
    gbc = nc.declare_dram_parameter("gbc", [128, 4], F32, isOutput=False)
    out_p = nc.declare_dram_parameter("out_p", [N, C], F32, isOutput=True)

    with tile.TileContext(nc) as tc:
        with (
            tc.tile_pool(name="persist", bufs=1) as persist,
            tc.tile_pool(name="weights", bufs=1) as weights,
        ):
            # ---- persistent SBUF tensors ----
            # per-head transposed q (duplicated over both partition halves so
            # score matmuls can alternate PE quadrants) and k (j-halves
            # stacked: rows 0:64 = j in [0,N/2), rows 64:128 = j in [N/2,N))
            # head-interleaved so phase-B copies cover several heads per
            # instruction; q head order is (q0, q2, q1) -- see QI
            TQall = persist.tile([128, H, N], BF16, tag="TQall")
            TKall = persist.tile([128, H, N // 2], BF16, tag="TKall")
            # e4m3 V for DoubleRow PV: [j, head, pair, slot, 80] where cols
            # 0:64 = v, 64 = ones (denominator), 65:80 = zero pad so the
            # slot stride is 16B-aligned. Pair cp covers chunks (cp, cp+16)
            # -- one from each j-half, matching the score quadrant alternation
            vA8 = persist.tile([128, H, CP, 2, 80], E4, tag="vA8")
            # attention output, channel-major: ao1 rows = h0,h1; ao2 rows = h2
            ao1 = persist.tile([128, N], F32R, tag="ao1")
            ao2 = persist.tile([64, N], F32R, tag="ao2")
            # per-channel v-quantization compensation, col h = corr_d/N
            corrT = persist.tile([64, H], F32, tag="corrT")
            onesb = persist.tile([128, 1], BF16, tag="onesb")

            ident = persist.tile([128, 128], F32, tag="ident")
            make_identity(nc, ident[:])
            identb = persist.tile([128, 128], BF16, tag="identb")
            nc.vector.tensor_copy(identb[:], ident[:])
            nc.vector.memset(vA8[:], 0.0)
            nc.vector.memset(vA8[:, :, :, :, 64:65], 1.0)
            nc.vector.memset(onesb[:], 1.0)

            wq = weights.tile([128, 6, J], BF16, tag="wqkv")
            wq_src = wqkv_t.rearrange("(ck p) j -> p ck j", p=128)
            nc.sync.dma_start(wq[:, 0, :], wq_src[:, 0, :])
            gbct = weights.tile([128, 4], F32, tag="gbc")
            epst = weights.tile([128, 1], F32, tag="epst")
            nc.vector.memset(epst[:], EPS)
            cbias = weights.tile([128, 1], F32, tag="cbias")
            nc.vector.memset(cbias[:], -C_BIAS)
            pw128 = weights.tile([128, C], F32R, tag="pw128")
            pw64 = weights.tile([64, C], F32R, tag="pw64")
            corrS = weights.tile([1, H * D], F32, tag="corrS")

            # ================= Phase B: qkv + LN + transpose =================
            # software pipeline, slot s handles: qkv(s), stats(s-1), apply(s-2),
            # transpose+copies(s-3)
            with (
                tc.tile_pool(name="pB", bufs=3) as pB,
                tc.tile_pool(name="pBs", bufs=4) as pBs,
                tc.tile_pool(name="psQ", bufs=2, space="PSUM") as psQ,
                tc.tile_pool(name="psT", bufs=2, space="PSUM") as psT,
                tc.tile_pool(name="psCp", bufs=1, space="PSUM") as psCp,
            ):
                psC = psCp.tile([1, H * D], F32, tag="psC")
                st = {}   # per-nb dict of live tiles

                def emit_qkv(nb):
                    xt = pB.tile([128, 6, 128], BF16, tag="xt")
                    nc.sync.dma_start(
                        xt[:],
                        x_t.rearrange("(ck p) n -> p ck n", p=128)[
                            :, :, nb * 128 : (nb + 1) * 128
                        ],
                    )
                    if nb == 0:
                        # remaining weight slices land while the first x tile
                        # is being consumed
                        for ck in range(1, 6):
                            nc.sync.dma_start(wq[:, ck, :], wq_src[:, ck, :])
                        nc.sync.dma_start(gbct[:], gbc[:, :])
                    # q|k at cols 0:384 (bank 0), v at 512:704 (bank 1)
                    ps = psQ.tile([128, 1024], F32, tag="qkvps")
                    for off, woff, w in ((0, 0, 384), (512, 384, 192)):
                        for ck in range(6):
                            nc.tensor.matmul(
                                ps[:, off : off + w],
                                xt[:, ck, :],
                                wq[:, ck, woff : woff + w],
                                start=(ck == 0),
                                stop=(ck == 5),
                            )
                    st[nb] = {"ps": ps}

                def emit_stats(nb):
                    t = st[nb]
                    ps = t.pop("ps")
                    qkvS = pBs.tile([128, G, D], BF16, tag="qkvS")
                    nc.vector.tensor_copy(
                        qkvS[:], ps[:, 0 : G * D].rearrange("p (g d) -> p g d", d=D)
                    )
                    vsrc = ps[:, 512 : 512 + H * D].rearrange("p (h d) -> p h d", d=D)
                    vdst = vA8[:, :, nb % CP, nb // CP, 0:64]
                    nc.scalar.copy(vdst, vsrc)
                    # v-quantization residual, reduced over j via a ones-matmul
                    rv = pBs.tile([128, H, D], BF16, tag="rv")
                    nc.vector.tensor_sub(rv[:], vsrc, vdst)
                    nc.tensor.matmul(
                        psC[0:1, :], onesb[:, 0:1], rv[:],
                        start=(nb == 0), stop=(nb == NB - 1),
                    )
                    s1 = pBs.tile([128, G], F32, tag="s1")
                    nc.vector.tensor_reduce(s1[:], qkvS[:], AXL.X, ALU.add)
                    mu = pBs.tile([128, G], F32, tag="mu")
                    nc.vector.tensor_scalar_mul(mu[:], s1[:], 1.0 / D)
                    sq = pBs.tile([128, G, D], BF16, tag="sq")
                    nc.vector.tensor_mul(sq[:], qkvS[:], qkvS[:])
                    s2 = pBs.tile([128, G], F32, tag="s2")
                    nc.vector.tensor_reduce(s2[:], sq[:], AXL.X, ALU.add)
                    musq = pBs.tile([128, G], F32, tag="musq")
                    nc.vector.tensor_mul(musq[:], mu[:], mu[:])
                    var = pBs.tile([128, G], F32, tag="var")
                    nc.vector.scalar_tensor_tensor(
                        var[:], s2[:], 1.0 / D, musq[:], ALU.mult, ALU.subtract
                    )
                    std = pBs.tile([128, G], F32, tag="std")
                    nc.scalar.activation(std[:], var[:], ACTF.Sqrt, bias=epst[:])
                    rstd = pBs.tile([128, G], F32, tag="rstd")
                    nc.vector.reciprocal_approx_fast(rstd[:], std[:])
                    t.update(qkvS=qkvS, mu=mu, rstd=rstd)

                def emit_apply(nb):
                    t = st[nb]
                    qkvS, mu, rstd = t.pop("qkvS"), t.pop("mu"), t.pop("rstd")
                    # cs = (x - mu) * rstd on gpsimd; gamma/beta are applied
                    # later, inside the post-transpose per-partition copies
                    cst = pBs.tile([128, G, D], BF16, tag="cst")
                    nc.gpsimd.tensor_sub(
                        cst[:], qkvS[:], mu[:, :, None].broadcast_to([128, G, D])
                    )
                    cs = pBs.tile([128, G, D], BF16, tag="cs")
                    nc.gpsimd.tensor_mul(
                        cs[:], cst[:], rstd[:, :, None].broadcast_to([128, G, D])
                    )
                    t["cs"] = cs

                def emit_transp(nb):
                    t = st[nb]
                    cs = t.pop("cs")
                    jh = nb // CP
                    cb = nb % CP
                    pst = psT.tile([128, 5, 128], BF16, tag="pst")
                    # q0|q1 packed in one transpose; q2 single; k singles land
                    # directly in their j-half rows via tile_position
                    nc.tensor.transpose(pst[:, 0, :], cs[:, 0:2, :], identb[:])
                    nc.tensor.transpose(pst[0:64, 1, :], cs[:, 2, :], identb[:])
                    for hh in range(H):
                        nc.tensor.transpose(
                            pst[64 * jh : 64 * jh + 64, 2 + hh, :],
                            cs[:, 3 + hh, :], identb[:],
                            tile_position=(0, 64 * jh),
                        )
                    blk = slice(nb * 128, (nb + 1) * 128)
                    kblk = slice(cb * 128, (cb + 1) * 128)
                    rows = slice(64 * jh, 64 * jh + 64)
                    # copies apply gamma/beta per partition (= per d after
                    # the transpose): out = in * gamma_col + beta_col.
                    # q0 & q2 land in adjacent head slots with one copy
                    nc.scalar.activation(
                        TQall[0:64, 0:2, blk], pst[0:64, 0:2, :], ACTF.Identity,
                        bias=gbct[0:64, 1:2], scale=gbct[0:64, 0:1],
                    )
                    nc.scalar.activation(
                        TQall[64:128, 2, blk], pst[64:128, 0, :], ACTF.Identity,
                        bias=gbct[64:128, 1:2], scale=gbct[64:128, 0:1],
                    )
                    nc.scalar.activation(
                        TKall[rows, :, kblk], pst[rows, 2:5, :], ACTF.Identity,
                        bias=gbct[rows, 3:4], scale=gbct[rows, 2:3],
                    )
                    # duplicate q into the other partition half
                    nc.sync.dma_start(TQall[64:128, 0:2, blk], TQall[0:64, 0:2, blk])
                    nc.sync.dma_start(TQall[0:64, 2, blk], TQall[64:128, 2, blk])
                    del st[nb]

                for s in range(NB + 3):
                    if s == 3:
                        nc.sync.dma_start(pw128[:], projw_t[0:128, :].bitcast(F32R))
                        nc.sync.dma_start(pw64[:], projw_t[128:192, :].bitcast(F32R))
                    if s >= 2 and s - 2 < NB:
                        emit_apply(s - 2)
                    if s >= 3:
                        emit_transp(s - 3)
                    if s < NB:
                        emit_qkv(s)
                    if s >= 1 and s - 1 < NB:
                        emit_stats(s - 1)

                # fold 1/N into the compensation and scatter to per-partition
                # columns (col h = corr for head h's 64 d's)
                nc.vector.tensor_scalar_mul(corrS[:], psC[0:1, :], 1.0 / N)
                for hh in range(H):
                    nc.sync.dma_start(
                        corrT[:, hh : hh + 1], corrS[0:1, hh * D : (hh + 1) * D]
                    )

            # ================= Phase C: attention + proj =================
            with (
                tc.tile_pool(name="pt", bufs=12) as ptp,
                tc.tile_pool(name="pCs", bufs=6) as pCs,
                tc.tile_pool(name="pD", bufs=3) as pD,
                tc.tile_pool(name="psS", bufs=3, space="PSUM") as psS,
                tc.tile_pool(name="psOD", bufs=2, space="PSUM") as psOD,
            ):
                pvq = []      # pending PV closures, global FIFO across heads
                side = []     # pending (ib, proj-unit) from the previous i-block
                ao_done = set()   # i-blocks whose h2 normalize has been emitted

                def pop_pv():
                    pvq.pop(0)()

                def pop_side():
                    if side and side[0][0] in ao_done:
                        side.pop(0)[1]()

                def normalize(ib, h, pso):
                    isl = slice(ib * 512, (ib + 1) * 512)
                    rden_f = pCs.tile([1, 512], F32, tag="rden_f")
                    nc.vector.tensor_copy(rden_f[:], pso[64:65, :])
                    rden = pCs.tile([1, 512], F32, tag="rden")
                    nc.vector.reciprocal_approx_fast(rden[:], rden_f[:])
                    rb = pCs.tile([64, 512], F32, tag="rb")
                    nc.gpsimd.partition_broadcast(rb[:], rden[:])
                    cr = corrT[:, h : h + 1]
                    if h == 0:
                        nc.vector.tensor_mul(ao1[0:64, isl], pso[0:64, :], rb[:])
                        nc.vector.tensor_scalar_add(ao1[0:64, isl], ao1[0:64, isl], cr)
                    elif h == 2:
                        nc.vector.tensor_mul(ao2[0:64, isl], pso[0:64, :], rb[:])
                        nc.vector.tensor_scalar_add(ao2[0:64, isl], ao2[0:64, isl], cr)
                        ao_done.add(ib)
                    else:
                        stg = pCs.tile([64, 512], F32R, tag="stg")
                        nc.vector.tensor_mul(stg[:], pso[0:64, :], rb[:])
                        nc.vector.tensor_scalar_add(stg[:], stg[:], cr)
                        nc.sync.dma_start(ao1[64:128, isl], stg[:])

                def make_pv(pso, h, cp, ib, pt):
                    def run():
                        nc.tensor.matmul(
                            pso,
                            vA8[:, h, cp, :, :],
                            pt[:].rearrange("p (s i) -> p s i", s=2),
                            start=(cp == 0),
                            stop=(cp == CP - 1),
                            perf_mode=DR,
                        )
                        if cp == CP - 1:
                            normalize(ib, h, pso)
                    return run

                def make_proj(ib):
                    units = []
                    for nb in range(ib * 4, ib * 4 + 4):
                        blk = slice(nb * 128, (nb + 1) * 128)
                        stage = [None]

                        def u1(blk=blk, stage=stage):
                            stage[0] = pD.tile([128, C], F32, tag="stage", name="stage")
                            pd_t = psOD.tile([128, 512], F32, tag="psod", name="pd")
                            nc.tensor.matmul(
                                pd_t[:, 0:512], r32(ao1[:, blk]),
                                r32(pw128[:, 0:512]), start=True, stop=False,
                            )
                            nc.tensor.matmul(
                                pd_t[:, 0:512], r32(ao2[0:64, blk]),
                                r32(pw64[0:64, 0:512]), start=False, stop=True,
                            )
                            nc.vector.tensor_copy(stage[0][:, 0:512], pd_t[:, 0:512])

                        def u2(blk=blk, stage=stage):
                            pd_t = psOD.tile([128, 512], F32, tag="psod", name="pd")
                            nc.tensor.matmul(
                                pd_t[:, 0:256], r32(ao1[:, blk]),
                                r32(pw128[:, 512:768]), start=True, stop=False,
                            )
                            nc.tensor.matmul(
                                pd_t[:, 0:256], r32(ao2[0:64, blk]),
                                r32(pw64[0:64, 512:768]), start=False, stop=True,
                            )
                            nc.vector.tensor_copy(stage[0][:, 512:768], pd_t[:, 0:256])
                            nc.sync.dma_start(out_p[blk, :], stage[0][:])

                        units.append((ib, u1))
                        units.append((ib, u2))
                    return units

                for ib in range(IB):
                    isl = slice(ib * 512, (ib + 1) * 512)
                    for h in range(H):
                        qi = QI[h]
                        pso_t = psOD.tile([128, 512], F32, tag="psod", name="pso")
                        pso = pso_t[0:80, :]
                        for cp in range(CP):
                            psSp = psS.tile([128, 1024], F32, tag="st")
                            # pair = chunks (cp, cp+16): one from each j-half,
                            # so consecutive score matmuls alternate quadrants
                            # and LDWEIGHTS hides behind the running stream
                            kblk = slice(cp * 128, (cp + 1) * 128)
                            nc.tensor.matmul(
                                psSp[:, 0:512],
                                TKall[0:64, h, kblk], TQall[0:64, qi, isl],
                                start=True, stop=True, tile_position=(0, 0),
                            )
                            if len(pvq) > SKEW_PAIRS:
                                pop_pv()
                            if cp % 2 == 0:
                                pop_side()
                            nc.tensor.matmul(
                                psSp[:, 512:1024],
                                TKall[64:128, h, kblk], TQall[64:128, qi, isl],
                                start=True, stop=True, tile_position=(64, 0),
                            )
                            if len(pvq) > SKEW_PAIRS + 1:
                                pop_pv()
                            pt = ptp.tile([128, 1024], E4, tag="pt")
                            if cp in SCHR_CPS:
                                nc.vector.tensor_scalar(
                                    pt[:].bitcast(U8), psSp[:], B_MAGIC, 0.0,
                                    ALU.add, ALU.max,
                                )
                            else:
                                nc.scalar.activation(
                                    pt[:], psSp[:], ACTF.Exp,
                                    bias=cbias[:], scale=1.0 / SCL,
                                )
                            pvq.append(make_pv(pso, h, cp, ib, pt))
                    # queue this i-block's projection for the next i-block
                    side.extend(make_proj(ib))
                    if ib == IB - 1:
                        while pvq:
                            pop_pv()
                        while side:
                            side.pop(0)[1]()

    nc.compile()
    return nc


@lru_cache(maxsize=2)
def _built(N):
    nc = build_nc(N)
    return nc


def _prep_inputs(x, qkv_w, q_gamma, q_beta, k_gamma, k_beta, proj_w):
    x = np.asarray(x, np.float32)
    qkv_w = np.asarray(qkv_w, np.float32)
    proj_w = np.asarray(proj_w, np.float32)
    B = x.shape[0]
    import ml_dtypes
    xts = [np.ascontiguousarray(x[b].T).astype(ml_dtypes.bfloat16) for b in range(B)]
    qsc = SCALE * SCL
    gb2 = np.stack(
        [
            np.tile(np.asarray(q_gamma, np.float32) * qsc, 2),
            np.tile(np.asarray(q_beta, np.float32) * qsc, 2),
            np.tile(np.asarray(k_gamma, np.float32), 2),
            np.tile(np.asarray(k_beta, np.float32), 2),
        ],
        axis=1,
    )  # [128, 4]
    gbs = []
    wqs = []
    pws = []
    for g in range(4):
        r = slice(192 * g, 192 * (g + 1))
        wq_rows = np.concatenate(
            [qkv_w[r], qkv_w[768:1536][r], qkv_w[1536:2304][r]], axis=0
        )
        wqs.append(np.ascontiguousarray(wq_rows.T).astype(ml_dtypes.bfloat16))
        pws.append(np.ascontiguousarray(proj_w[:, r].T))
        gbs.append(gb2)
    in_maps = []
    for core in range(8):
        b, g = core // 4, core % 4
        in_maps.append(
            {"x_t": xts[b], "wqkv_t": wqs[g], "projw_t": pws[g], "gbc": gbs[g]}
        )
    return in_maps


def run_cores(in_maps, N, trace=False):
    from concourse.bass_utils import run_bass_kernel_spmd

    nc = _built(N)
    res = run_bass_kernel_spmd(nc, in_maps, list(range(8)), trace=trace)
    return res


def kernel(x, qkv_w, q_gamma, q_beta, k_gamma, k_beta, proj_w, proj_b):
    x = np.asarray(x, np.float32)
    N = x.shape[1]
    in_maps = _prep_inputs(x, qkv_w, q_gamma, q_beta, k_gamma, k_beta, proj_w)
    res = run_cores(in_maps, N)
    parts = [np.asarray(r["out_p"], np.float32) for r in res.results]
    out0 = parts[0] + parts[1] + parts[2] + parts[3]
    out1 = parts[4] + parts[5] + parts[6] + parts[7]
    out = np.stack([out0, out1]) + np.asarray(proj_b, np.float32)
    return out.astype(np.float32)


# revision 25
# speedup vs baseline: 1.2265x; 1.0209x over previous
"""Trainium2 Bass kernel for CustomAttention (qkv -> per-head LN on q,k -> SDPA -> proj).

Sharding: 8 cores = 2 batches x 4 head-groups (3 heads each).
Per core: qkv projection for its heads from x[b], full attention per head,
then a partial output projection over its 192 channels. Host sums the 4
partials per batch and adds proj_b.

Key numerics/scheduling (vs the bf16 baseline):
 - PV matmuls run in fp8-e4m3 DoubleRow mode: V (+ ones column for the
   softmax denominator) is quantized to e4m3 and stored with two adjacent
   128-j chunks in the DoubleRow slot dim, so each PV matmul contracts
   256 j at 0.5 cycles/row -- ~4x fewer PE cycles than the bf16 PV.
 - A v-quantization mean-compensation term (colsum(v - e4m3(v))/N, added
   per-channel after normalize) claws back accuracy.
 - Scores stay bf16, but q is pre-scaled by 8/ln2 so the probability
   tiles can be produced two ways at identical scale: scalar engine
   activation Exp (scale=ln2/8, bias=-2.75, e4m3 out) for 11/16 pairs,
   and a one-op Schraudolph exp on the vector engine (x + magic, max 0,
   uint8 out bitcast as e4m3) for 5/16 pairs -- splitting the 50M-element
   exp stream across two engines. |logit| <= |q||k| = 8 (LN guarantees
   the norms) bounds the uint8 codes to [0, 126], so no saturation edge
   cases.
 - Phase B transposes are packed two heads per [128,128] transpose and
   the old quadrant-pair q duplication is dropped (trace showed quadrant
   matmuls serialize anyway).
"""

import os
import sys
from functools import lru_cache

import numpy as np

for _p in ("/opt/trn_rl_repo", os.path.expanduser("~/.axon_site/_ro/trn_rl_repo")):
    if os.path.isdir(_p) and _p not in sys.path:
        sys.path.insert(0, _p)

import concourse.bass as bass
import concourse.mybir as mybir
from concourse import bacc
import concourse.tile as tile
from concourse.masks import make_identity

F32 = mybir.dt.float32
F32R = mybir.dt.float32r
BF16 = mybir.dt.bfloat16
E4 = mybir.dt.float8e4
U8 = mybir.dt.uint8
ALU = mybir.AluOpType
ACTF = mybir.ActivationFunctionType
AXL = mybir.AxisListType
DR = mybir.MatmulPerfMode.DoubleRow

H = 3          # heads per core
D = 64         # head dim
C = 768        # model dim
J = 3 * H * D  # qkv rows per core = 576
G = 2 * H      # merged LN virtual heads (q0..2, k0..2)
EPS = 1e-5
SCALE = D ** -0.5
SCL = 8.0 / float(np.log(2.0))      # folded into q; scores come out as l*SCL
C_BIAS = 2.75                        # softmax bias: p = exp(l - C_BIAS)
M_SHIFT = 0.4639                     # schraudolph mantissa-correction shift
# f32->uint8 conversion on the DVE rounds to nearest, so no +0.5
B_MAGIC = 56.0 - SCL * C_BIAS - M_SHIFT
SCHR_CPS = frozenset((1, 4, 6, 9, 11, 14))  # pairs routed to the DVE schr-exp
QI = (0, 2, 1)  # head h lives at TQall[:, QI[h], :] (q2 packed next to q0)
SKEW_PAIRS = 7


def r32(ap):
    return ap.bitcast(F32R)


def build_nc(N=4096):
    """One-core program; all 8 cores run it SPMD with different input data."""
    NB = N // 128          # j-chunks
    CP = NB // 2           # DoubleRow chunk pairs
    IB = N // 512          # i-blocks

    nc = bacc.Bacc("TRN2", target_bir_lowering=False, debug=False)
    x_t = nc.declare_dram_parameter("x_t", [C, N], BF16, isOutput=False)
    wqkv_t = nc.declare_dram_parameter("wqkv_t", [C, J], BF16, isOutput=False)
    projw_t = nc.declare_dram_parameter("projw_t", [H * D, C], F32, isOutput=False)
    # per-partition LN affine columns, rows = d duplicated over both halves:
    # col 0 = gamma_q*scale*SCL, 1 = beta_q*scale*SCL, 2 = gamma_k, 3 = beta_k
    gbc = nc.declare_dram_parameter("gbc", [128, 4], F32, isOutput=False)
    out_p = nc.declare_dram_parameter("out_p", [N, C], F32, isOutput=True)

    with tile.TileContext(nc) as tc:
        with (
            tc.tile_pool(name="persist", bufs=1) as persist,
            tc.tile_pool(name="weights", bufs=1) as weights,
        ):
            # ---- persistent SBUF tensors ----
            # per-head transposed q (duplicated over both partition halves so
            # score matmuls can alternate PE quadrants) and k (j-halves
            # stacked: rows 0:64 = j in [0,N/2), rows 64:128 = j in [N/2,N))
            # head-interleaved so phase-B copies cover several heads per
            # instruction; q head order is (q0, q2, q1) -- see QI
            TQall = persist.tile([128, H, N], BF16, tag="TQall")
            TKall = persist.tile([128, H, N // 2], BF16, tag="TKall")
            # e4m3 V for DoubleRow PV: [j, head, pair, slot, 80] where cols
            # 0:64 = v, 64 = ones (denominator), 65:80 = zero pad so the
            # slot stride is 16B-aligned. Pair cp covers chunks (cp, cp+16)
            # -- one from each j-half, matching the score quadrant alternation
            vA8 = persist.tile([128, H, CP, 2, 80], E4, tag="vA8")
            # attention output, channel-major: ao1 rows = h0,h1; ao2 rows = h2
            ao1 = persist.tile([128, N], F32R, tag="ao1")
            ao2 = persist.tile([64, N], F32R, tag="ao2")
            # per-channel v-quantization compensation, col h = corr_d/N
            corrT = persist.tile([64, H], F32, tag="corrT")
            onesb = persist.tile([128, 1], BF16, tag="onesb")

            ident = persist.tile([128, 128], F32, tag="ident")
            make_identity(nc, ident[:])
            identb = persist.tile([128, 128], BF16, tag="identb")
            nc.vector.tensor_copy(identb[:], ident[:])
            nc.vector.memset(vA8[:], 0.0)
            nc.vector.memset(vA8[:, :, :, :, 64:65], 1.0)
            nc.vector.memset(onesb[:], 1.0)

            wq = weights.tile([128, 6, J], BF16, tag="wqkv")
            wq_src = wqkv_t.rearrange("(ck p) j -> p ck j", p=128)
            nc.sync.dma_start(wq[:, 0, :], wq_src[:, 0, :])
            gbct = weights.tile([128, 4], F32, tag="gbc")
            epst = weights.tile([128, 1], F32, tag="epst")
            nc.vector.memset(epst[:], EPS)
            cbias = weights.tile([128, 1], F32, tag="cbias")
            nc.vector.memset(cbias[:], -C_BIAS)
            pw128 = weights.tile([128, C], F32R, tag="pw128")
            pw64 = weights.tile([64, C], F32R, tag="pw64")
            corrS = weights.tile([1, H * D], F32, tag="corrS")

            # ================= Phase B: qkv + LN + transpose =================
            # software pipeline, slot s handles: qkv(s), stats(s-1), apply(s-2),
            # transpose+copies(s-3)
            with (
                tc.tile_pool(name="pB", bufs=3) as pB,
                tc.tile_pool(name="pBs", bufs=4) as pBs,
                tc.tile_pool(name="psQ", bufs=2, space="PSUM") as psQ,
                tc.tile_pool(name="psT", bufs=2, space="PSUM") as psT,
                tc.tile_pool(name="psCp", bufs=1, space="PSUM") as psCp,
            ):
                psC = psCp.tile([1, H * D], F32, tag="psC")
                st = {}   # per-nb dict of live tiles

                def emit_qkv(nb):
                    xt = pB.tile([128, 6, 128], BF16, tag="xt")
                    nc.sync.dma_start(
                        xt[:],
                        x_t.rearrange("(ck p) n -> p ck n", p=128)[
                            :, :, nb * 128 : (nb + 1) * 128
                        ],
                    )
                    if nb == 0:
                        # remaining weight slices land while the first x tile
                        # is being consumed
                        for ck in range(1, 6):
                            nc.sync.dma_start(wq[:, ck, :], wq_src[:, ck, :])
                        nc.sync.dma_start(gbct[:], gbc[:, :])
                    # q|k at cols 0:384 (bank 0), v at 512:704 (bank 1)
                    ps = psQ.tile([128, 1024], F32, tag="qkvps")
                    for off, woff, w in ((0, 0, 384), (512, 384, 192)):
                        for ck in range(6):
                            nc.tensor.matmul(
                                ps[:, off : off + w],
                                xt[:, ck, :],
                                wq[:, ck, woff : woff + w],
                                start=(ck == 0),
                                stop=(ck == 5),
                            )
                    st[nb] = {"ps": ps}

                def emit_stats(nb):
                    t = st[nb]
                    ps = t.pop("ps")
                    qkvS = pBs.tile([128, G, D], BF16, tag="qkvS")
                    nc.vector.tensor_copy(
                        qkvS[:], ps[:, 0 : G * D].rearrange("p (g d) -> p g d", d=D)
                    )
                    vsrc = ps[:, 512 : 512 + H * D].rearrange("p (h d) -> p h d", d=D)
                    vdst = vA8[:, :, nb % CP, nb // CP, 0:64]
                    nc.scalar.copy(vdst, vsrc)
                    # v-quantization residual, reduced over j via a ones-matmul
                    rv = pBs.tile([128, H, D], BF16, tag="rv")
                    nc.vector.tensor_sub(rv[:], vsrc, vdst)
                    nc.tensor.matmul(
                        psC[0:1, :], onesb[:, 0:1], rv[:],
                        start=(nb == 0), stop=(nb == NB - 1),
                    )
                    s1 = pBs.tile([128, G], F32, tag="s1")
                    nc.vector.tensor_reduce(s1[:], qkvS[:], AXL.X, ALU.add)
                    mu = pBs.tile([128, G], F32, tag="mu")
                    nc.gpsimd.tensor_scalar_mul(mu[:], s1[:], 1.0 / D)
                    sq = pBs.tile([128, G, D], BF16, tag="sq")
                    nc.vector.tensor_mul(sq[:], qkvS[:], qkvS[:])
                    s2 = pBs.tile([128, G], F32, tag="s2")
                    nc.vector.tensor_reduce(s2[:], sq[:], AXL.X, ALU.add)
                    musq = pBs.tile([128, G], F32, tag="musq")
                    nc.gpsimd.tensor_mul(musq[:], mu[:], mu[:])
                    var = pBs.tile([128, G], F32, tag="var")
                    nc.vector.scalar_tensor_tensor(
                        var[:], s2[:], 1.0 / D, musq[:], ALU.mult, ALU.subtract
                    )
                    std = pBs.tile([128, G], F32, tag="std")
                    nc.scalar.activation(std[:], var[:], ACTF.Sqrt, bias=epst[:])
                    rstd = pBs.tile([128, G], F32, tag="rstd")
                    nc.vector.reciprocal_approx_fast(rstd[:], std[:])
                    t.update(qkvS=qkvS, mu=mu, rstd=rstd)

                def emit_apply(nb):
                    t = st[nb]
                    qkvS, mu, rstd = t.pop("qkvS"), t.pop("mu"), t.pop("rstd")
                    # cs = (x - mu) * rstd on gpsimd; gamma/beta are applied
                    # later, inside the post-transpose per-partition copies
                    cst = pBs.tile([128, G, D], BF16, tag="cst")
                    nc.gpsimd.tensor_sub(
                        cst[:], qkvS[:], mu[:, :, None].broadcast_to([128, G, D])
                    )
                    cs = pBs.tile([128, G, D], BF16, tag="cs")
                    nc.gpsimd.tensor_mul(
                        cs[:], cst[:], rstd[:, :, None].broadcast_to([128, G, D])
                    )
                    t["cs"] = cs

                def emit_transp(nb):
                    t = st[nb]
                    cs = t.pop("cs")
                    jh = nb // CP
                    cb = nb % CP
                    pst = psT.tile([128, 5, 128], BF16, tag="pst")
                    # q0|q1 packed in one transpose; q2 single; k singles land
                    # directly in their j-half rows via tile_position
                    nc.tensor.transpose(pst[:, 0, :], cs[:, 0:2, :], identb[:])
                    nc.tensor.transpose(pst[0:64, 1, :], cs[:, 2, :], identb[:])
                    for hh in range(H):
                        nc.tensor.transpose(
                            pst[64 * jh : 64 * jh + 64, 2 + hh, :],
                            cs[:, 3 + hh, :], identb[:],
                            tile_position=(0, 64 * jh),
                        )
                    blk = slice(nb * 128, (nb + 1) * 128)
                    kblk = slice(cb * 128, (cb + 1) * 128)
                    rows = slice(64 * jh, 64 * jh + 64)
                    # copies apply gamma/beta per partition (= per d after
                    # the transpose): out = in * gamma_col + beta_col.
                    # q0 & q2 land in adjacent head slots with one copy
                    nc.scalar.activation(
                        TQall[0:64, 0:2, blk], pst[0:64, 0:2, :], ACTF.Identity,
                        bias=gbct[0:64, 1:2], scale=gbct[0:64, 0:1],
                    )
                    nc.scalar.activation(
                        TQall[64:128, 2, blk], pst[64:128, 0, :], ACTF.Identity,
                        bias=gbct[64:128, 1:2], scale=gbct[64:128, 0:1],
                    )
                    nc.scalar.activation(
                        TKall[rows, :, kblk], pst[rows, 2:5, :], ACTF.Identity,
                        bias=gbct[rows, 3:4], scale=gbct[rows, 2:3],
                    )
                    # duplicate q into the other partition half
                    nc.sync.dma_start(TQall[64:128, 0:2, blk], TQall[0:64, 0:2, blk])
                    nc.sync.dma_start(TQall[0:64, 2, blk], TQall[64:128, 2, blk])
                    del st[nb]

                for s in range(NB + 3):
                    if s == 3:
                        nc.sync.dma_start(pw128[:], projw_t[0:128, :].bitcast(F32R))
                        nc.sync.dma_start(pw64[:], projw_t[128:192, :].bitcast(F32R))
                    if s >= 2 and s - 2 < NB:
                        emit_apply(s - 2)
                    if s >= 3:
                        emit_transp(s - 3)
                    if s < NB:
                        emit_qkv(s)
                    if s >= 1 and s - 1 < NB:
                        emit_stats(s - 1)

                # fold 1/N into the compensation and scatter to per-partition
                # columns (col h = corr for head h's 64 d's)
                nc.vector.tensor_scalar_mul(corrS[:], psC[0:1, :], 1.0 / N)
                for hh in range(H):
                    nc.sync.dma_start(
                        corrT[:, hh : hh + 1], corrS[0:1, hh * D : (hh + 1) * D]
                    )

            # ================= Phase C: attention + proj =================
            with (
                tc.tile_pool(name="pt", bufs=12) as ptp,
                tc.tile_pool(name="pCs", bufs=6) as pCs,
                tc.tile_pool(name="pD", bufs=3) as pD,
                tc.tile_pool(name="psS", bufs=3, space="PSUM") as psS,
                tc.tile_pool(name="psOD", bufs=2, space="PSUM") as psOD,
            ):
                pvq = []      # pending PV closures, global FIFO across heads
                side = []     # pending (ib, proj-unit) from the previous i-block
                ao_done = set()   # i-blocks whose h2 normalize has been emitted

                def pop_pv():
                    pvq.pop(0)()

                def pop_side():
                    if side and side[0][0] in ao_done:
                        side.pop(0)[1]()

                def normalize(ib, h, pso):
                    isl = slice(ib * 512, (ib + 1) * 512)
                    rden_f = pCs.tile([1, 512], F32, tag="rden_f")
                    nc.vector.tensor_copy(rden_f[:], pso[64:65, :])
                    rden = pCs.tile([1, 512], F32, tag="rden")
                    nc.vector.reciprocal_approx_fast(rden[:], rden_f[:])
                    rb = pCs.tile([64, 512], F32, tag="rb")
                    nc.gpsimd.partition_broadcast(rb[:], rden[:])
                    cr = corrT[:, h : h + 1]
                    if h == 0:
                        nc.vector.tensor_mul(ao1[0:64, isl], pso[0:64, :], rb[:])
                        nc.vector.tensor_scalar_add(ao1[0:64, isl], ao1[0:64, isl], cr)
                    elif h == 2:
                        nc.vector.tensor_mul(ao2[0:64, isl], pso[0:64, :], rb[:])
                        nc.vector.tensor_scalar_add(ao2[0:64, isl], ao2[0:64, isl], cr)
                        ao_done.add(ib)
                    else:
                        stg = pCs.tile([64, 512], F32R, tag="stg")
                        nc.vector.tensor_mul(stg[:], pso[0:64, :], rb[:])
                        nc.vector.tensor_scalar_add(stg[:], stg[:], cr)
                        nc.sync.dma_start(ao1[64:128, isl], stg[:])

                def make_pv(pso, h, cp, ib, pt):
                    def run():
                        nc.tensor.matmul(
                            pso,
                            vA8[:, h, cp, :, :],
                            pt[:].rearrange("p (s i) -> p s i", s=2),
                            start=(cp == 0),
                            stop=(cp == CP - 1),
                            perf_mode=DR,
                        )
                        if cp == CP - 1:
                            normalize(ib, h, pso)
                    return run

                def make_proj(ib):
                    units = []
                    for nb in range(ib * 4, ib * 4 + 4):
                        blk = slice(nb * 128, (nb + 1) * 128)
                        stage = [None]

                        def u1(blk=blk, stage=stage):
                            stage[0] = pD.tile([128, C], F32, tag="stage", name="stage")
                            pd_t = psOD.tile([128, 512], F32, tag="psod", name="pd")
                            nc.tensor.matmul(
                                pd_t[:, 0:512], r32(ao1[:, blk]),
                                r32(pw128[:, 0:512]), start=True, stop=False,
                            )
                            nc.tensor.matmul(
                                pd_t[:, 0:512], r32(ao2[0:64, blk]),
                                r32(pw64[0:64, 0:512]), start=False, stop=True,
                            )
                            nc.vector.tensor_copy(stage[0][:, 0:512], pd_t[:, 0:512])

                        def u2(blk=blk, stage=stage):
                            pd_t = psOD.tile([128, 512], F32, tag="psod", name="pd")
                            nc.tensor.matmul(
                                pd_t[:, 0:256], r32(ao1[:, blk]),
                                r32(pw128[:, 512:768]), start=True, stop=False,
                            )
                            nc.tensor.matmul(
                                pd_t[:, 0:256], r32(ao2[0:64, blk]),
                                r32(pw64[0:64, 512:768]), start=False, stop=True,
                            )
                            nc.vector.tensor_copy(stage[0][:, 512:768], pd_t[:, 0:256])
                            nc.sync.dma_start(out_p[blk, :], stage[0][:])

                        units.append((ib, u1))
                        units.append((ib, u2))
                    return units

                for ib in range(IB):
                    isl = slice(ib * 512, (ib + 1) * 512)
                    for h in range(H):
                        qi = QI[h]
                        pso_t = psOD.tile([128, 512], F32, tag="psod", name="pso")
                        pso = pso_t[0:80, :]
                        for cp in range(CP):
                            psSp = psS.tile([128, 1024], F32, tag="st")
                            # pair = chunks (cp, cp+16): one from each j-half,
                            # so consecutive score matmuls alternate quadrants
                            # and LDWEIGHTS hides behind the running stream
                            kblk = slice(cp * 128, (cp + 1) * 128)
                            nc.tensor.matmul(
                                psSp[:, 0:512],
                                TKall[0:64, h, kblk], TQall[0:64, qi, isl],
                                start=True, stop=True, tile_position=(0, 0),
                            )
                            if len(pvq) > SKEW_PAIRS:
                                pop_pv()
                            if cp % 2 == 0:
                                pop_side()
                            nc.tensor.matmul(
                                psSp[:, 512:1024],
                                TKall[64:128, h, kblk], TQall[64:128, qi, isl],
                                start=True, stop=True, tile_position=(64, 0),
                            )
                            if len(pvq) > SKEW_PAIRS + 1:
                                pop_pv()
                            pt = ptp.tile([128, 1024], E4, tag="pt")
                            if cp in SCHR_CPS:
                                nc.vector.tensor_scalar(
                                    pt[:].bitcast(U8), psSp[:], B_MAGIC, 0.0,
                                    ALU.add, ALU.max,
                                )
                            else:
                                nc.scalar.activation(
                                    pt[:], psSp[:], ACTF.Exp,
                                    bias=cbias[:], scale=1.0 / SCL,
                                )
                            pvq.append(make_pv(pso, h, cp, ib, pt))
                    # queue this i-block's projection for the next i-block
                    side.extend(make_proj(ib))
                    if ib == IB - 1:
                        while pvq:
                            pop_pv()
                        while side:
                            side.pop(0)[1]()

    nc.compile()
    return nc


@lru_cache(maxsize=2)
def _built(N):
    nc = build_nc(N)
    return nc


def _prep_inputs(x, qkv_w, q_gamma, q_beta, k_gamma, k_beta, proj_w):
    x = np.asarray(x, np.float32)
    qkv_w = np.asarray(qkv_w, np.float32)
    proj_w = np.asarray(proj_w, np.float32)
    B = x.shape[0]
    import ml_dtypes
    xts = [np.ascontiguousarray(x[b].T).astype(ml_dtypes.bfloat16) for b in range(B)]
    qsc = SCALE * SCL
    gb2 = np.stack(
        [
            np.tile(np.asarray(q_gamma, np.float32) * qsc, 2),
            np.tile(np.asarray(q_beta, np.float32) * qsc, 2),
            np.tile(np.asarray(k_gamma, np.float32), 2),
            np.tile(np.asarray(k_beta, np.float32), 2),
        ],
        axis=1,
    )  # [128, 4]
    gbs = []
    wqs = []
    pws = []
    for g in range(4):
        r = slice(192 * g, 192 * (g + 1))
        wq_rows = np.concatenate(
            [qkv_w[r], qkv_w[768:1536][r], qkv_w[1536:2304][r]], axis=0
        )
        wqs.append(np.ascontiguousarray(wq_rows.T).astype(ml_dtypes.bfloat16))
        pws.append(np.ascontiguousarray(proj_w[:, r].T))
        gbs.append(gb2)
    in_maps = []
    for core in range(8):
        b, g = core // 4, core % 4
        in_maps.append(
            {"x_t": xts[b], "wqkv_t": wqs[g], "projw_t": pws[g], "gbc": gbs[g]}
        )
    return in_maps


def run_cores(in_maps, N, trace=False):
    from concourse.bass_utils import run_bass_kernel_spmd

    nc = _built(N)
    res = run_bass_kernel_spmd(nc, in_maps, list(range(8)), trace=trace)
    return res


def kernel(x, qkv_w, q_gamma, q_beta, k_gamma, k_beta, proj_w, proj_b):
    x = np.asarray(x, np.float32)
    N = x.shape[1]
    in_maps = _prep_inputs(x, qkv_w, q_gamma, q_beta, k_gamma, k_beta, proj_w)
    res = run_cores(in_maps, N)
    parts = [np.asarray(r["out_p"], np.float32) for r in res.results]
    out0 = parts[0] + parts[1] + parts[2] + parts[3]
    out1 = parts[4] + parts[5] + parts[6] + parts[7]
    out = np.stack([out0, out1]) + np.asarray(proj_b, np.float32)
    return out.astype(np.float32)
